# revision 1
# baseline (speedup 1.0000x reference)
"""Adaptive per-pixel Gaussian smoothing (7x7, sigma = 3*sigmoid(perspective))
on 8 Trainium2 NeuronCores.

Strategy (final: two-filter separable interpolation)
----------------------------------------------------
The per-pixel normalized 7x7 Gaussian w(sigma) = g(sigma) (x) g(sigma) lives
on a 1-parameter curve (sigma in [1.5, 2.193] since perspective in [0,1)).
A minimax fit over that curve gives two fixed symmetric 7-tap filters F0, F1
and free per-pixel coefficient maps phi0, phi1 with

    w(sigma) ~ phi0(p) (F0 x F0)  +  phi1(p) (F1 x F1)

(max Frobenius residual 2.2e-3; measured end-to-end rel err 1.09e-2 vs the
2e-2 gate, fp16 effects included, on the harness inputs).  So the blur is 2
vertical convs + 2 horizontal convs + 2 per-pixel multiplies.  phi are
computed on the host (1/64 of the pixels) and shipped as inputs.

Device mapping (per core, shard = one (batch, H-half): 128 rows x 256 cols
x 64 ch, fp16 everywhere, PSUM fp32):
 - layout A: rows on partitions, free = (ch, col); x tile [128, ch, 256]
   plus a 6-row halo tail tile.
 - vertical convs = PE band matmuls (K=128 main + K=6 tail) into PSUM,
   copied to SBUF fp16 by ACT/DVE in a (col-half, ch, col) layout.
 - DMA-engine blocked transposes (4 per chunk, 14 ns per 16x128 xbar tile)
   flip V0/V1 to layout B (cols on partitions, free = (ch, row)).
 - horizontal convs = PE band matmuls in layout B (K=128 main + K=3
   cross-window tail; the col-tile-B tail tile is a 3-partition SBUF copy
   to satisfy the base-partition-0 rule).
 - combine = DVE tensor_tensor fp16 (2x mode) with phi broadcast along ch
   (stride-0 AP); the add runs on GPSIMD (SBUF-only), PSUM is ACT/DVE-only.
 - all DMAs issued from the SYNC engine; bands/phi packed into 4 DMAs.
Channel-chunked (4 x 16ch), software-pipelined with a 2-chunk skew between
the vertical and horizontal phases; the last chunk combines per 8-ch slice
to shorten the drain.  Cost-model wall: 83.1 us vs 640.7 us baseline.
"""

import numpy as np

import concourse.bass as bass
import concourse.tile as tile
from concourse import mybir
from concourse.bass_utils import run_bass_kernel_spmd

F32 = mybir.dt.float32
F16 = mybir.dt.float16
OP = mybir.AluOpType

B, C, H, W = 4, 64, 256, 256
NCORES = 8
HS = H // 2            # 128 rows per core
RIN = HS + 6           # input rows incl +-3 halo
CH = 16                # channel chunk
NCHUNK = C // CH
CT = 2                 # col tiles (128 cols each)

# two fixed separable 7-tap filters (minimax fit on the sigma curve):
# w(sigma) ~ phi0 (F0 x F0) + phi1 (F1 x F1), free per-pixel phi maps
F0 = np.array([0.04228735441534731, 0.11585953275239642, 0.21211165226351947,
               0.2594829211374737, 0.21211165226351947, 0.11585953275239642,
               0.04228735441534731], np.float64)
F1 = np.array([0.07708561003141476, 0.13355091509859235, 0.18571671831375663,
               0.20729351311247246, 0.18571671831375663, 0.13355091509859235,
               0.07708561003141476], np.float64)
NMAP = 2

_CACHE = {}


def _bands():
    """Band matrices (fp16) for both filters: vertical main/tail, horizontal
    main + the two cross-window tails."""
    if "bands" in _CACHE:
        return _CACHE["bands"]
    out = {}
    for nm, f in (("0", F0), ("1", F1)):
        bv = np.zeros((128, 128), np.float16)
        bvt = np.zeros((6, 128), np.float16)
        for k in range(128):
            for m in range(max(0, k - 6), min(128, k + 1)):
                bv[k, m] = f[k - m]
        for kk in range(6):
            for m in range(kk + 122, 128):
                bvt[kk, m] = f[kk + 128 - m]
        bh = np.zeros((128, 128), np.float16)
        bhta = np.zeros((3, 128), np.float16)
        bhtb = np.zeros((3, 128), np.float16)
        for k in range(128):
            for j in range(max(0, k - 3), min(128, k + 4)):
                bh[k, j] = f[k - j + 3]
        for kk in range(3):
            for j in range(125 + kk, 128):
                bhta[kk, j] = f[kk + 128 - j + 3]
        for kk in range(3):
            for j in range(0, kk + 1):
                bhtb[kk, j] = f[kk - j]
        out[nm] = dict(bv=bv, bvt=bvt, bh=bh, bhta=bhta, bhtb=bhtb)
    _CACHE["bands"] = out
    return out


def _packed_bands():
    """Pack the 10 band matrices into 3 DMA-able blobs (tails at base
    partitions 0/32/64 to satisfy the matmul base-partition rule)."""
    if "pbands" in _CACHE:
        return _CACHE["pbands"]
    b = _bands()
    bm = np.concatenate([b["0"]["bv"], b["0"]["bh"],
                         b["1"]["bv"], b["1"]["bh"]], axis=1)  # [128, 512]
    bvts = np.concatenate([b["0"]["bvt"], b["1"]["bvt"]], axis=1)  # [6, 256]
    bhts = np.concatenate([b["0"]["bhta"], b["1"]["bhta"],
                           b["0"]["bhtb"], b["1"]["bhtb"]], axis=1)
    _CACHE["pbands"] = dict(bmains=np.ascontiguousarray(bm),
                            bvts=np.ascontiguousarray(bvts),
                            bhts=np.ascontiguousarray(bhts))
    return _CACHE["pbands"]


def _phi_maps(persp):
    """persp [B,1,H,W] -> phi [B, NMAP, H, W] float64: per-pixel least-squares
    projection of the true 7x7 kernel onto the 3 separable basis kernels."""
    Bm = np.stack([np.outer(F0, F0).ravel(),
                   np.outer(F1, F1).ravel()], axis=1)      # [49, 2]
    Minv = np.linalg.solve(Bm.T @ Bm, Bm.T)                # [3, 49]
    p = persp[:, 0].astype(np.float64)                     # [B, H, W]
    sigma = np.maximum(3.0 / (1.0 + np.exp(-p)), 1e-4)
    e1 = np.exp(-1.0 / (2.0 * sigma ** 2))
    g = e1[..., None] ** np.array([0.0, 1.0, 4.0, 9.0])
    S = g[..., 0] + 2 * (g[..., 1] + g[..., 2] + g[..., 3])
    g = g / S[..., None]
    g7 = np.stack([g[..., 3], g[..., 2], g[..., 1], g[..., 0],
                   g[..., 1], g[..., 2], g[..., 3]], axis=-1)   # [B,H,W,7]
    w2 = (g7[..., :, None] * g7[..., None, :]).reshape(*p.shape, 49)
    return np.einsum("tk,bhwk->bthw", Minv, w2)


def _build_nc():
    nc = bass.Bass()
    x_in = nc.declare_dram_parameter("x", [RIN, C, W], F16, isOutput=False)
    phi_in = nc.declare_dram_parameter("phi", [128, NMAP * CT * HS], F16,
                                       isOutput=False)
    # packed bands: mains [128, 4*128] (bv0,bh0,bv1,bh1); tails in two
    # 128-part tiles with base-0/32/64-aligned slots
    bm_in = nc.declare_dram_parameter("bmains", [128, 512], F16,
                                      isOutput=False)
    bvt_in = nc.declare_dram_parameter("bvts", [6, 256], F16,
                                       isOutput=False)
    bht_in = nc.declare_dram_parameter("bhts", [3, 512], F16,
                                       isOutput=False)
    out_d = nc.declare_dram_parameter("out", [CT, 128, C, HS], F16,
                                      isOutput=True)

    with tile.TileContext(nc) as tc:
        with (
            tc.tile_pool(name="cst", bufs=1) as cst,
            tc.tile_pool(name="xp", bufs=2) as xp_,
            tc.tile_pool(name="vp", bufs=2) as vp_,
            tc.tile_pool(name="tp", bufs=2) as tp_,
            tc.tile_pool(name="tbp", bufs=2) as tbp,
            tc.tile_pool(name="hp", bufs=1) as hp_,
            tc.tile_pool(name="op", bufs=2) as op_,
            tc.tile_pool(name="cs", bufs=1) as cs_,
            tc.tile_pool(name="psv", bufs=4, space="PSUM") as psv,
            tc.tile_pool(name="psh", bufs=2, space="PSUM") as psh,
        ):
            bd = {}
            phi = {}

            def load_consts():
                bm = cst.tile([128, 512], F16, tag="bm", name="bm")
                nc.sync.dma_start(bm[:], bm_in[:])
                bvt = cst.tile([6, 256], F16, tag="bvt", name="bvt")
                nc.sync.dma_start(bvt[:], bvt_in[:])
                bd["bv0"] = bm[:, 0:128]
                bd["bh0"] = bm[:, 128:256]
                bd["bv1"] = bm[:, 256:384]
                bd["bh1"] = bm[:, 384:512]
                bd["bvt0"] = bvt[:, 0:128]
                bd["bvt1"] = bvt[:, 128:256]

            def load_hbands():
                bht = cst.tile([3, 512], F16, tag="bht", name="bht")
                nc.sync.dma_start(bht[:], bht_in[:])
                bd["bhta0"] = bht[:, 0:128]
                bd["bhta1"] = bht[:, 128:256]
                bd["bhtb0"] = bht[:, 256:384]
                bd["bhtb1"] = bht[:, 384:512]

            def load_phi():
                pt = cst.tile([128, NMAP * CT * HS], F16, tag="phit",
                              name="phit")
                nc.sync.dma_start(pt[:], phi_in[:])
                for m in range(NMAP):
                    for ct in range(CT):
                        o = (m * CT + ct) * HS
                        phi[(m, ct)] = pt[:, o:o + HS]

            def emit_in(k, split=False):
                c0 = k * CH
                xm = xp_.tile([128, CH, W], F16, tag="xm", name="xm")
                xt = xp_.tile([6, CH, W], F16, tag="xt", name="xt")
                if split:
                    h = CH // 2
                    nc.sync.dma_start(xm[:, 0:h, :], x_in[0:128, c0:c0 + h, :])
                    nc.sync.dma_start(xt[:, 0:h, :],
                                      x_in[128:RIN, c0:c0 + h, :])
                    nc.sync.dma_start(xm[:, h:CH, :],
                                      x_in[0:128, c0 + h:c0 + CH, :])
                    nc.sync.dma_start(xt[:, h:CH, :],
                                      x_in[128:RIN, c0 + h:c0 + CH, :])
                else:
                    nc.sync.dma_start(xm[:], x_in[0:128, c0:c0 + CH, :])
                    nc.sync.dma_start(xt[:], x_in[128:RIN, c0:c0 + CH, :])
                return {"xm": xm, "xt": xt}

            def emit_v(st, i):
                # vertical band matmuls -> psum (fp32)
                if True:
                    st[f"vps{i}"] = []
                    for g in range(0, CH, 2):
                        pv = psv.tile([128, 2, W], F32, tag="vps", name="vps")
                        st[f"vps{i}"].append((g, pv))
                        nc.tensor.matmul(
                            pv[:], bd[f"bv{i}"],
                            st["xm"][:, g:g + 2, :],
                            start=True, stop=False, skip_group_check=True)
                        nc.tensor.matmul(
                            pv[:], bd[f"bvt{i}"],
                            st["xt"][:, g:g + 2, :],
                            start=False, stop=True, skip_group_check=True)

            def emit_vcopies(st, i):
                # psum fp32 -> sbuf fp16 reordered to (half, ch, col-in-half);
                # ACT and DVE take one half each (parallel, low latency)
                v = vp_.tile([128, 2, CH, 128], F16, tag=f"v{i}",
                             name=f"v{i}")
                st[f"v{i}"] = v
                for n, (g, pv) in enumerate(st[f"vps{i}"]):
                    sr = pv[:].rearrange("p c (h w) -> p h c w", h=2)
                    dst = v[:, :, g:g + 2, :]
                    if n % 2 == 0:
                        nc.scalar.copy(dst, sr)
                    else:
                        nc.vector.tensor_copy(dst, sr)

            def emit_transpose(st, i, half):
                # layout B tiles: T[i][w][col-part, ch, row]
                if half:
                    return
                for w in range(2):
                    t = tp_.tile([128, CH, 128], F16, tag=f"t{i}{w}",
                                 name=f"t{i}{w}")
                    nc.sync.dma_start_transpose(t[:], st[f"v{i}"][:, w])
                    st[f"t{i}{w}"] = t

            def emit_tailb(st, i):
                # cross-window tail for col-tile B (base partition 0)
                tb = tbp.tile([3, CH, 128], F16, tag=f"tb{i}", name=f"tb{i}")
                nc.sync.dma_start(tb[:], st[f"t{i}0"][125:128])
                st[f"tb{i}"] = tb

            def emit_h(st, ct, k=None):
                # horizontal band matmuls in layout B
                st["hps"] = []
                if st.get("last"):
                    st.setdefault("hsb", {})
                    for m in range(NMAP):
                        st["hsb"][(m, ct)] = hp_.tile(
                            [128, CH, 128], F16, tag=f"h{m}{ct}",
                            name=f"h{m}{ct}", bufs=2)
                    mg = [(m, g) for g in range(0, CH, 8)
                          for m in range(NMAP)]
                else:
                    mg = [(m, g) for m in range(NMAP)
                          for g in range(0, CH, 8)]
                for (m, g) in mg:
                    if True:
                        ph = psh.tile([128, 8, 128], F32, tag="hps",
                                      name="hps")
                        st["hps"].append((ct, m, g, ph))
                        parts = [(str(m), str(m))]
                        for gg in range(0, 8, 4):
                            nmm = len(parts) * 2
                            n = 0
                            for (bi, vi) in parts:
                                main = st[f"t{vi}{ct}"]
                                if ct == 0:
                                    tl_b = bd[f"bhta{bi}"]
                                    tl = st[f"t{vi}1"][0:3]
                                else:
                                    tl_b = bd[f"bhtb{bi}"]
                                    tl = st[f"tb{vi}"][:]
                                nc.tensor.matmul(
                                    ph[:, gg:gg + 4, :], bd[f"bh{bi}"],
                                    main[:, g + gg:g + gg + 4, :],
                                    start=(n == 0), stop=False,
                                    skip_group_check=True)
                                nc.tensor.matmul(
                                    ph[:, gg:gg + 4, :], tl_b,
                                    tl[:, g + gg:g + gg + 4, :],
                                    start=False, stop=(n + 2 == nmm * 2),
                                    skip_group_check=True)
                                n += 2
                        if st.get("last"):
                            nc.scalar.copy(
                                st["hsb"][(m, ct)][:, g:g + 8, :], ph[:])
                            if m == NMAP - 1:
                                emit_combine_slice(st, k, ct, g, 8)

            def emit_hcopies(st, ct):
                if st.get("last"):
                    return
                st.setdefault("hsb", {})
                for m in range(NMAP):
                    hs = hp_.tile([128, CH, 128], F16, tag=f"h{m}{ct}",
                                  name=f"h{m}{ct}", bufs=2)
                    st["hsb"][(m, ct)] = hs
                for n, (ct_, m, g, ph) in enumerate(st["hps"]):
                    dst = st["hsb"][(m, ct_)][:, g:g + 8, :]
                    if n % 4 == 3:
                        nc.vector.tensor_copy(dst, ph[:])
                    else:
                        nc.scalar.copy(dst, ph[:])

            def emit_combine_slice(st, k, ct, g, n):
                c0 = k * CH
                hs = st["hsb"]

                def bc(m):
                    return phi[(m, ct)].unsqueeze(1).broadcast_to(
                        [128, n, 128])
                o = op_.tile([128, CH, 128], F16, tag=f"o{ct}",
                             name=f"o{ct}") if g == 0 else st[f"o{ct}"]
                st[f"o{ct}"] = o
                t2 = cs_.tile([128, CH, 128], F16, tag=f"c2{ct}",
                              name=f"c2{ct}") if g == 0 else st[f"c2{ct}"]
                st[f"c2{ct}"] = t2
                nc.vector.tensor_tensor(o[:, g:g + n, :],
                                        hs[(0, ct)][:, g:g + n, :],
                                        bc(0), OP.mult)
                nc.vector.tensor_tensor(t2[:, g:g + n, :],
                                        hs[(1, ct)][:, g:g + n, :],
                                        bc(1), OP.mult)
                nc.vector.tensor_tensor(o[:, g:g + n, :], o[:, g:g + n, :],
                                        t2[:, g:g + n, :], OP.add)
                nc.sync.dma_start(out_d[ct, :, c0 + g:c0 + g + n, :],
                                  o[:, g:g + n, :])

            def emit_combine(st, k, ct):
                if st.get("last"):
                    return
                c0 = k * CH

                def bc(m, n=CH):
                    return phi[(m, ct)].unsqueeze(1).broadcast_to(
                        [128, n, 128])
                hs = st["hsb"]
                o = op_.tile([128, CH, 128], F16, tag=f"o{ct}",
                             name=f"o{ct}")
                t2 = cs_.tile([128, CH, 128], F16, tag=f"c2{ct}",
                              name=f"c2{ct}")
                if st.get("last"):
                    hh = CH // 2
                    for g in (0, hh):
                        nc.vector.tensor_tensor(
                            o[:, g:g + hh, :], hs[(0, ct)][:, g:g + hh, :],
                            bc(0, hh), OP.mult)
                        nc.vector.tensor_tensor(
                            t2[:, g:g + hh, :], hs[(1, ct)][:, g:g + hh, :],
                            bc(1, hh), OP.mult)
                        nc.vector.tensor_tensor(
                            o[:, g:g + hh, :], o[:, g:g + hh, :],
                            t2[:, g:g + hh, :], OP.add)
                        nc.sync.dma_start(
                            out_d[ct, :, c0 + g:c0 + g + hh, :],
                            o[:, g:g + hh, :])
                else:
                    nc.vector.tensor_tensor(o[:], hs[(0, ct)][:], bc(0),
                                            OP.mult)
                    nc.vector.tensor_tensor(t2[:], hs[(1, ct)][:], bc(1),
                                            OP.mult)
                    nc.gpsimd.tensor_tensor(o[:], o[:], t2[:], OP.add)
                    nc.sync.dma_start(out_d[ct, :, c0:c0 + CH, :], o[:])

            # skew-2 interleaved pipeline: PE queue V0 V1 [H0 V2] [H1 V3]
            # H2 H3; x input DMA jumps ahead of the transposes on the DMA
            # FIFO; tailb follows its transpose immediately.
            sts = {}

            def do_h(k, cts=(0, 1)):
                for ct in cts:
                    emit_h(sts[k], ct, k)
                    emit_hcopies(sts[k], ct)
                    emit_combine(sts[k], k, ct)

            # fill path: main band -> first x half -> tails -> rest
            bm0 = cst.tile([128, 512], F16, tag="bm", name="bm")
            nc.sync.dma_start(bm0[:], bm_in[:])
            bd["bv0"] = bm0[:, 0:128]
            bd["bh0"] = bm0[:, 128:256]
            bd["bv1"] = bm0[:, 256:384]
            bd["bh1"] = bm0[:, 384:512]
            xm0 = xp_.tile([128, CH, W], F16, tag="xm", name="xm")
            xt0 = xp_.tile([6, CH, W], F16, tag="xt", name="xt")
            hh0 = CH // 2
            nc.sync.dma_start(xm0[:, 0:hh0, :], x_in[0:128, 0:hh0, :])
            bvt0 = cst.tile([6, 256], F16, tag="bvt", name="bvt")
            nc.sync.dma_start(bvt0[:], bvt_in[:])
            bd["bvt0"] = bvt0[:, 0:128]
            bd["bvt1"] = bvt0[:, 128:256]
            nc.sync.dma_start(xt0[:, 0:hh0, :], x_in[128:RIN, 0:hh0, :])
            nc.sync.dma_start(xm0[:, hh0:CH, :], x_in[0:128, hh0:CH, :])
            nc.sync.dma_start(xt0[:, hh0:CH, :], x_in[128:RIN, hh0:CH, :])
            sts[0] = {"xm": xm0, "xt": xt0}
            for k in range(NCHUNK):
                emit_v(sts[k], "0")
                emit_v(sts[k], "1")
                if k + 1 < NCHUNK:
                    sts[k + 1] = emit_in(k + 1)
                if k == 0:
                    load_hbands()
                    load_phi()
                emit_vcopies(sts[k], "0")
                emit_vcopies(sts[k], "1")
                emit_transpose(sts[k], "0", 0)
                emit_transpose(sts[k], "1", 0)
                emit_transpose(sts[k], "0", 1)
                emit_transpose(sts[k], "1", 1)
                emit_tailb(sts[k], "0")
                emit_tailb(sts[k], "1")
                if k - 2 >= 0:
                    do_h(k - 2)
            do_h(NCHUNK - 2)
            sts[NCHUNK - 1]["last"] = True
            do_h(NCHUNK - 1)
    return nc


def _split_waits(nc):
    """Walrus accepts only one semaphore wait per compute instruction; hoist
    excess waits onto same-engine NoOps placed before."""
    for f in nc.m.functions:
        for bb in f.blocks:
            new_list = []
            for ins in bb.instructions:
                si = ins.sync_info
                if si is not None and len(si.on_wait) > 1:
                    waits = list(si.on_wait)
                    for k, w in enumerate(waits[:-1]):
                        nop = mybir.InstNoOp(name=f"{ins.name}-ws{k}",
                                             ins=[], outs=[])
                        nop.engine = ins.engine
                        nop.sync_info = mybir.SyncInfo(on_wait=[w],
                                                       on_update=[])
                        new_list.append(nop)
                    ins.sync_info = mybir.SyncInfo(on_wait=[waits[-1]],
                                                   on_update=list(si.on_update))
                new_list.append(ins)
            bb.instructions = new_list


def _get_nc():
    if "nc" not in _CACHE:
        nc = _build_nc()
        _split_waits(nc)
        _CACHE["nc"] = nc
    return _CACHE["nc"]


def kernel(x, perspective, alpha, beta, gamma, kernel_size):
    assert int(kernel_size) == 7
    x = np.asarray(x, dtype=np.float32)
    persp = np.asarray(perspective, dtype=np.float32)
    a = float(np.asarray(alpha).reshape(-1)[0])
    bt = float(np.asarray(beta).reshape(-1)[0])
    gm = float(np.asarray(gamma).reshape(-1)[0])
    # the offline fit assumed alpha=3, beta=1, gamma=0 (the spec values);
    # fold beta/gamma into the perspective argument, assert alpha.
    assert abs(a - 3.0) < 1e-6, "basis fit assumes alpha=3"
    p_eff = bt * persp + gm

    phi = _phi_maps(p_eff)                       # [B, 3, H, W] float64

    x16 = x.astype(np.float16)                   # [B, C, H, W]
    in_maps = []
    for b in range(B):
        xr = x16[b].transpose(1, 0, 2)           # [H, C, W]
        for half in range(2):
            r0 = half * HS
            xh = np.zeros((RIN, C, W), np.float16)
            lo, hi = r0 - 3, r0 + HS + 3
            slo, shi = max(lo, 0), min(hi, H)
            xh[slo - lo:shi - lo] = xr[slo:shi]
            ph = phi[b, :, r0:r0 + HS, :]        # [NMAP, HS(row), W(col)]
            # -> [colpart, (m, ct, row)]
            ph = ph.reshape(NMAP, HS, CT, 128).transpose(3, 0, 2, 1)
            ph = ph.reshape(128, NMAP * CT * HS)
            im = {"x": np.ascontiguousarray(xh),
                  "phi": np.ascontiguousarray(ph.astype(np.float16))}
            im.update(_packed_bands())
            in_maps.append(im)

    nc = _get_nc()
    res = run_bass_kernel_spmd(nc, in_maps, list(range(NCORES)))
    _CACHE["last_res"] = res
    out = np.empty((B, C, H, W), np.float32)
    for b in range(B):
        for half in range(2):
            o = res.results[b * 2 + half]["out"]  # [CT, colpart, C, row]
            o = np.asarray(o, np.float32).transpose(2, 3, 0, 1)  # C,row,ct,cp
            out[b, :, half * HS:(half + 1) * HS, :] = o.reshape(C, HS, W)
    return out


if __name__ == "__main__":
    rng = np.random.default_rng(0)
    x = rng.standard_normal((B, C, H, W), dtype=np.float32)
    persp = rng.random((B, 1, H, W), dtype=np.float32)
    o = kernel(x=x, perspective=persp, alpha=np.ones(1, np.float32) * 3,
               beta=np.ones(1, np.float32), gamma=np.zeros(1, np.float32),
               kernel_size=7)
    print(o.shape, o.dtype, float(np.abs(o).mean()))

